# revision 37
# baseline (speedup 1.0000x reference)
"""NgramHasher Trainium2 kernel.

Computes h[b,s,ch] = (sum_j coeffs[k,j] * window_j) mod 2^20 for ngram sizes
(2, 3) x 8 tables, on 8 NeuronCores (data parallel over batch).

Math: with c = c0 + 2^10*c1 and t = t0 + 2^10*t1 (all chunks 10-bit),
  h = (A + 2^10 * (B mod 2^10)) mod 2^20
  A = sum_j c0[j]*t0[s-j]                     (fp16 matmul, exact in fp32 PSUM)
  B = sum_j (c0[j]*t1[s-j] + c1[j]*t0[s-j])   (fp16 matmul, K=20)
The B pass adds a 2^23 bias row so its fp32 PSUM bit pattern is
0x4B000000 + B (fixed exponent -> mantissa IS B). Per chunk:
  - ScalarE (closer to PSUM, otherwise idle) value-casts ps_a fp32 -> u32
    SBUF tile: exactly A (A < 2^22, no bias needed).
  - One DVE scalar_tensor_tensor fuses: out = (ps_b_bits << 22) XOR a_u32.
    0x4B000000 << 22 vanishes mod 2^32, so bits 22..31 = B mod 1024 and
    bits 0..21 = A, disjoint. (Two PSUM operands in one DVE op are
    impossible -- PSUM has a single DVE read port -- hence the ScalarE hop.)
The host decodes h = (A + 1024*(B mod 1024)) & 0xFFFFF. No identity-matmul
third pass: per chunk it's 4 matmuls + 1 ACT copy + 1 DVE op + 1 DMA.

Matmul packing: M = 128 = 8 position-offsets (g) x 16 channels; moving
columns are position groups q (position s = 8q + g). Weights are banded
Toeplitz. Device output is [128, 8192] uint32 per core (channel-planar);
the host unshards/permutes/decodes to [64, 8192, 16] int64.

Measured bottleneck on 8 concurrently-running cores is the OUT stream
(~250 GB/s/core effective, 4 MB/core), with PE a close second (pinned at
1.2 GHz here: 427ns per 512-col fp16 matmul; the HAM 2.4 GHz un-throttle
never engages in this environment, and bf16 measures identical). Hence:
all X DMA issues are hoisted ahead on alternating queues, the first and
last chunks run as 512-wide pieces so the OUT stream starts ~2us earlier
and drains ~1us sooner, and OUT ships from two queues.
"""
import sys
sys.path.insert(0, "/opt/trn_rl_repo")
import numpy as np
from contextlib import ExitStack
from numpy.lib.stride_tricks import sliding_window_view

import concourse.bass as bass
import concourse.tile as tile
from concourse import bacc, mybir
from concourse.bass_utils import run_bass_kernel_spmd

dt = mybir.dt
AluOp = mybir.AluOpType

N_CORES = 8
B, S = 64, 8192
B_LOC = B // N_CORES            # batch rows per core
P_CORE = B_LOC * S              # positions per core (65536)
G = 8                           # position offsets packed into M
NCH = 16                        # output channels (2 ngram sizes x 8 tables)
Q = P_CORE // G                 # moving columns per core (8192)
QCHUNK = 1024                   # columns per pipeline chunk (= 2 PSUM banks)
NCHUNK = Q // QCHUNK

_NC_CACHE = {}


def _stt_u32(eng, out, in0, imm, in1, op0, op1):
    """scalar_tensor_tensor with an integer-typed immediate.

    The bass wrapper lowers python-int scalars as float32 ImmediateValue,
    which the BIR verifier rejects for bitvec ops ("ImmVal must be integer
    and match the type of src and dst"). Build the instruction directly
    with a uint32 immediate instead.
    """
    return eng.add_instruction(
        mybir.InstTensorScalarPtr(
            name=eng.bass.get_next_instruction_name(),
            is_scalar_tensor_tensor=True,
            op0=op0,
            op1=op1,
            ins=[eng.lower_ap(in0),
                 mybir.ImmediateValue(dtype=dt.uint32, value=imm),
                 eng.lower_ap(in1)],
            outs=[eng.lower_ap(out)],
        ))


def _build_bass():
    """Build the SPMD Bass program (identical on all 8 cores)."""
    nc = bacc.Bacc("TRN2", target_bir_lowering=False, debug=False,
                   num_devices=N_CORES)
    x_d = nc.dram_tensor("X", [21, Q], dt.float16, kind="ExternalInput").ap()
    w_d = nc.dram_tensor("WAB", [21, 256], dt.float16, kind="ExternalInput").ap()
    out_d = nc.dram_tensor("OUT", [128, Q], dt.uint32, kind="ExternalOutput").ap()

    with tile.TileContext(nc) as tc:
        with ExitStack() as ctx:
            # bufs > NCHUNK: no SBUF slot is ever reused, so DMAs carry no
            # WAR waits (walrus "Too many sync wait commands" otherwise).
            wpool = ctx.enter_context(tc.tile_pool(name="w", bufs=1))
            xpool = ctx.enter_context(tc.tile_pool(name="x", bufs=NCHUNK + 1))
            apool = ctx.enter_context(tc.tile_pool(name="a", bufs=NCHUNK + 1))
            opool = ctx.enter_context(tc.tile_pool(name="o", bufs=NCHUNK + 1))
            psa = ctx.enter_context(tc.tile_pool(name="psa", bufs=2, space="PSUM"))
            psb = ctx.enter_context(tc.tile_pool(name="psb", bufs=2, space="PSUM"))

            # The stream phase is PE-production-bound (~300 GB/s equivalent)
            # with DMA headroom (~350 GB/s peak observed), so everything aims
            # at starting PE and the OUT stream as early as possible.
            # Weights first (they gate the first matmul), then chunk 0 as
            # 256/256/512 tiles so the first OUT data exists ~1us sooner.
            w_ab = wpool.tile([21, 256], dt.float16, tag="wab")
            nc.sync.dma_start(w_ab[:], w_d[:])
            w_a = w_ab[:, 0:128]
            w_b = w_ab[:, 128:256]
            x0a = xpool.tile([21, 256], dt.float16, tag="xq")
            nc.sync.dma_start(x0a[:], x_d[:, 0:256])
            x0b = xpool.tile([21, 256], dt.float16, tag="xq")
            nc.sync.dma_start(x0b[:], x_d[:, 256:512])
            x0c = xpool.tile([21, 512], dt.float16, tag="xh")
            nc.scalar.dma_start(x0c[:], x_d[:, 512:1024])
            xts = [None]
            for ci in range(1, NCHUNK):
                xt = xpool.tile([21, QCHUNK], dt.float16, tag="xt")
                # Alternate queues: the serial ~740ns/issue on one queue
                # would stretch the X window into the OUT stream.
                eng = nc.sync if ci % 2 else nc.scalar
                eng.dma_start(xt[:], x_d[:, ci * QCHUNK:(ci + 1) * QCHUNK])
                xts.append([xt[:, 0:512], xt[:, 512:1024]])

            def piece(q0, w, xh, out_eng):
                """Narrow A/B/copy/combine/store pipeline piece."""
                ps_a = psa.tile([128, w], dt.float32, tag="psa", name="pap")
                nc.tensor.matmul(ps_a[:], w_a, xh, start=True, stop=True)
                ps_b = psb.tile([128, w], dt.float32, tag="psb", name="pbp")
                nc.tensor.matmul(ps_b[:], w_b, xh, start=True, stop=True)
                a_sb = apool.tile([128, w], dt.uint32, tag="a", name="ap")
                nc.scalar.copy(a_sb[:], ps_a[:])
                o = opool.tile([128, w], dt.uint32, tag="o", name="op")
                _stt_u32(nc.vector, o[:], ps_b[:].bitcast(dt.uint32), 22,
                         a_sb[:], AluOp.logical_shift_left, AluOp.bitwise_xor)
                out_eng.dma_start(out_d[:, q0:q0 + w], o[:])

            # First chunk in 256/256/512 pieces: the first OUT DMA starts
            # ~2.5us earlier, which matters because the stream is the
            # critical path from then on.
            piece(0, 256, x0a[:], nc.gpsimd)
            piece(256, 256, x0b[:], nc.gpsimd)
            piece(512, 512, x0c[:], nc.gpsimd)

            for ci in range(1, NCHUNK - 2):
                q0 = ci * QCHUNK
                xa, xb = xts[ci]
                # A pass first: the ScalarE copy of ps_a overlaps the B pass,
                # shortening the per-chunk (and final-chunk) critical path.
                ps_a = psa.tile([128, QCHUNK], dt.float32, tag="psa")
                nc.tensor.matmul(ps_a[:, 0:512], w_a, xa, start=True, stop=True)
                nc.tensor.matmul(ps_a[:, 512:1024], w_a, xb, start=True, stop=True)
                ps_b = psb.tile([128, QCHUNK], dt.float32, tag="psb")
                nc.tensor.matmul(ps_b[:, 0:512], w_b, xa, start=True, stop=True)
                nc.tensor.matmul(ps_b[:, 512:1024], w_b, xb, start=True, stop=True)

                a_sb = apool.tile([128, QCHUNK], dt.uint32, tag="a")
                nc.scalar.copy(a_sb[:], ps_a[:])
                # Full-width combine (a half-split STT invites the static
                # scheduler to interleave chunks on the Vector queue with
                # over-conservative waits); OUT still ships as two 512-wide
                # halves on two queues.
                o = opool.tile([128, QCHUNK], dt.uint32, tag="o")
                _stt_u32(nc.vector, o[:], ps_b[:].bitcast(dt.uint32), 22,
                         a_sb[:], AluOp.logical_shift_left, AluOp.bitwise_xor)
                nc.gpsimd.dma_start(out_d[:, q0:q0 + 512], o[:, 0:512])
                nc.sync.dma_start(out_d[:, q0 + 512:q0 + QCHUNK], o[:, 512:1024])

            # Last two chunks in independent 512-wide pieces: the final
            # combines and OUT DMAs hang off the final matmuls, and the
            # 512-wide psb ring slots recycle ~1.7us sooner than 1024-wide
            # ones would, removing the end-of-run PE stall. Same pool tags
            # (slot size is the ring max, so no extra PSUM banks).
            for ci in (NCHUNK - 2, NCHUNK - 1):
                q0 = ci * QCHUNK
                xa, xb = xts[ci]
                piece(q0, 512, xa, nc.gpsimd)
                piece(q0 + 512, 512, xb,
                      nc.sync if ci == NCHUNK - 1 else nc.gpsimd)
    nc.compile()
    return nc


def _get_nc():
    if "nc" not in _NC_CACHE:
        _NC_CACHE["nc"] = _build_bass()
    return _NC_CACHE["nc"]


def _band(cpart):
    """[8,3] coeff chunk -> banded Toeplitz [10, 128] weight (fp32 values)."""
    W = np.zeros((10, 128), np.float32)
    for g in range(G):
        for k in range(8):
            for j in range(2):              # ngram n=2 -> channels 0..7
                W[g + 1 + j, g * 16 + k] = cpart[k, j]
            for j in range(3):              # ngram n=3 -> channels 8..15
                W[g + j, g * 16 + 8 + k] = cpart[k, j]
    return W


def _host_prep(token_ids, coeffs):
    t = np.asarray(token_ids).astype(np.int64)
    c = np.asarray(coeffs).astype(np.int64)

    t0 = (t & 0x3FF).astype(np.float16)     # [64, 8192]
    t1 = (t >> 10).astype(np.float16)
    pad = np.zeros((B, 2), np.float16)
    t0p = np.concatenate([pad, t0], axis=1)  # [64, 8194]
    t1p = np.concatenate([pad, t1], axis=1)
    # w?[b, q_loc, r] = t?p[b, 8*q_loc + r],  q_loc in [0,1024), r in [0,10)
    w0 = sliding_window_view(t0p, 10, axis=1)[:, ::G, :]
    w1 = sliding_window_view(t1p, 10, axis=1)[:, ::G, :]
    w0 = np.ascontiguousarray(w0.transpose(0, 2, 1))  # [64, 10, 1024]
    w1 = np.ascontiguousarray(w1.transpose(0, 2, 1))

    c0 = (c & 0x3FF).astype(np.float32)
    c1 = (c >> 10).astype(np.float32)
    # 2^23 bias arrives as (2^15 weight) * (2^8 const input row): both fp16-exact
    bias_row = np.full((1, 128), float(1 << 15), np.float32)
    WB = np.concatenate([_band(c0), _band(c1), bias_row],
                        axis=0).astype(np.float16)
    WA = np.concatenate([np.zeros((10, 128), np.float32), _band(c0),
                         np.zeros((1, 128), np.float32)],
                        axis=0).astype(np.float16)
    WAB = np.concatenate([WA, WB], axis=1)       # [21, 256]

    in_maps = []
    for core in range(N_CORES):
        b0 = core * B_LOC
        X = np.empty((21, Q), np.float16)
        # rows 0..9: X1 windows; rows 10..19: X0 windows; row 20: const 1.0
        X[0:10] = w1[b0:b0 + B_LOC].transpose(1, 0, 2).reshape(10, Q)
        X[10:20] = w0[b0:b0 + B_LOC].transpose(1, 0, 2).reshape(10, Q)
        X[20] = 256.0
        in_maps.append({"X": X, "WAB": WAB})
    return in_maps


def _unshard(results):
    out = np.empty((B, S, NCH), np.int64)
    for core, res in enumerate(results):
        raw = res["OUT"]                         # [128, Q] u32
        # bits 0..21: A;  bits 22..31: B mod 1024
        a = raw & 0x3FFFFF
        m = raw >> 22
        o = (a + (m << 10)) & 0xFFFFF
        o = o.reshape(G, NCH, Q)                 # [g, ch, q]
        o = o.transpose(2, 0, 1).reshape(P_CORE, NCH)  # [8q+g, ch]
        out[core * B_LOC:(core + 1) * B_LOC] = \
            o.reshape(B_LOC, S, NCH).astype(np.int64)
    return out


def _run(token_ids, coeffs, **spmd_kwargs):
    in_maps = _host_prep(token_ids, coeffs)
    nc = _get_nc()
    res = run_bass_kernel_spmd(nc, in_maps, core_ids=list(range(N_CORES)),
                               **spmd_kwargs)
    return _unshard(res.results), res


def kernel(token_ids, coeffs):
    out, _ = _run(token_ids, coeffs)
    return out
